# revision 3
# baseline (speedup 1.0000x reference)
"""GAT 2-layer kernel for trn2, 8 NeuronCores (SPMD).

Strategy (self-contained, hardcoded for N=100000, E=1600000, F=300):
 - nodes sharded contiguously across 8 cores (12500 each), degree-sorted
   within each core into 128-node tiles with a per-tile padded degree G_t
   (shared profile across cores so one SPMD program serves all cores)
 - 3 device launches, all dense DMA + PE/DVE/ACT compute:
     A: h1 = x @ W1, e_src/e_dst attention logits        -> [12544, 80]/core
     B: layer-1 edge softmax + weighted sum + b1 + ELU + W2aug -> [12544,66]
     C: layer-2 edge softmax + weighted sum + b2 + log_softmax -> [12544,64]
 - between launches the HOST performs the per-edge row gathers (pure index
   reordering into the layout the device streams densely).  Softmax is
   computed without the segment-max shift (mathematically identical, values
   are small enough for fp32 exp).
"""

import sys

sys.path.insert(0, "/opt/trn_rl_repo")

import numpy as np

import concourse.bass as bass
import concourse.bacc as bacc
import concourse.tile as tile
from concourse import mybir
from concourse.bass_utils import run_bass_kernel_spmd
from concourse.masks import make_identity

P = 128
NCORES = 8
N = 100000
F_IN = 300
FK = 384  # F_IN padded to 3*128 for matmul K-chunking
NPC = N // NCORES          # 12500 real nodes per core
NPAD = 12544               # padded to 98 tiles of 128
NT = NPAD // P             # 98 tiles
SENT_BIG = -1.0e9          # e_src of the dummy table row

_cache = {}


# ---------------------------------------------------------------- host prep
def _host_prep(edge_index):
    src = np.asarray(edge_index[0], dtype=np.int64)
    dst = np.asarray(edge_index[1], dtype=np.int64)
    src = np.concatenate([src, np.arange(N, dtype=np.int64)])
    dst = np.concatenate([dst, np.arange(N, dtype=np.int64)])
    deg = np.bincount(dst, minlength=N)

    # CSR by dst
    order_e = np.argsort(dst, kind="stable")
    srcs_by_dst = src[order_e].astype(np.int64)
    row_ptr = np.zeros(N + 1, dtype=np.int64)
    np.cumsum(deg, out=row_ptr[1:])

    # per-core degree-sorted node order, padded with -1
    order_all = np.full((NCORES, NPAD), -1, dtype=np.int64)
    for c in range(NCORES):
        lo = c * NPC
        nodes = lo + np.argsort(deg[lo : lo + NPC], kind="stable")
        order_all[c, :NPC] = nodes

    # pi position of each node (row in the concatenated per-core shards)
    pos = np.empty(N + 1, dtype=np.int64)
    for c in range(NCORES):
        pos[order_all[c, :NPC]] = c * NPAD + np.arange(NPC)
    pos[N] = NCORES * NPAD  # sentinel -> dummy row appended to tables

    # shared tile degree profile
    degp = np.zeros((NCORES, NPAD), dtype=np.int64)
    for c in range(NCORES):
        degp[c, :NPC] = deg[order_all[c, :NPC]]
    G = degp.reshape(NCORES, NT, P).max(axis=(0, 2))
    G = np.maximum(G + (G & 1), 2).astype(np.int64)  # round up to even, >=2

    # slot->table-position map, per core, flat per-tile [P, G_t] blocks
    tot_slots = int((P * G).sum())
    A = np.full((NCORES, tot_slots), NCORES * NPAD, dtype=np.int64)
    toff = np.zeros(NT + 1, dtype=np.int64)
    np.cumsum(P * G, out=toff[1:])
    pos_by_dst = pos[srcs_by_dst]
    for c in range(NCORES):
        for t in range(NT):
            g = int(G[t])
            nodes = order_all[c, t * P : (t + 1) * P]
            safe = np.where(nodes >= 0, nodes, 0)
            k = np.where(nodes >= 0, deg[safe], 0)
            gi = np.arange(g)[None, :]
            mask = gi < k[:, None]
            src_idx = np.minimum(row_ptr[safe][:, None] + gi, len(pos_by_dst) - 1)
            blk = np.where(mask, pos_by_dst[src_idx], NCORES * NPAD)
            A[c, toff[t] : toff[t + 1]] = blk.ravel()
    return order_all, pos, G, A, tot_slots


# ------------------------------------------------------------- launch A prog
def _build_A():
    nc = bacc.Bacc(None, target_bir_lowering=False)
    xT = nc.dram_tensor("xT", [FK, NPAD], mybir.dt.float32, kind="ExternalInput")
    w1 = nc.dram_tensor("w1", [FK, 64], mybir.dt.float32, kind="ExternalInput")
    asrc = nc.dram_tensor("asrc", [64], mybir.dt.float32, kind="ExternalInput")
    adst = nc.dram_tensor("adst", [64], mybir.dt.float32, kind="ExternalInput")
    out = nc.dram_tensor("h1x", [NPAD, 80], mybir.dt.float32, kind="ExternalOutput")

    f32 = mybir.dt.float32
    with tile.TileContext(nc) as tc:
        with (
            tc.tile_pool(name="const", bufs=1) as cp,
            tc.tile_pool(name="xin", bufs=3) as xp,
            tc.tile_pool(name="work", bufs=3) as wp,
            tc.tile_pool(name="psum", bufs=2, space="PSUM") as pp,
        ):
            w1_t = cp.tile([P, 3, 64], f32)
            nc.sync.dma_start(
                out=w1_t[:], in_=w1[:, :].rearrange("(k p) n -> p k n", p=P)
            )
            asrc_t = cp.tile([P, 64], f32)
            nc.sync.dma_start(
                out=asrc_t[:],
                in_=bass.AP(tensor=asrc, offset=0, ap=[[0, P], [1, 64]]),
            )
            adst_t = cp.tile([P, 64], f32)
            nc.sync.dma_start(
                out=adst_t[:],
                in_=bass.AP(tensor=adst, offset=0, ap=[[0, P], [1, 64]]),
            )
            for t in range(NT):
                xt = xp.tile([P, 3, P], f32, tag="x")
                nc.sync.dma_start(
                    out=xt[:],
                    in_=bass.AP(
                        tensor=xT,
                        offset=t * P,
                        ap=[[NPAD, P], [NPAD * P, 3], [1, P]],
                    ),
                )
                h_ps = pp.tile([P, 64], f32, tag="h")
                for k in range(3):
                    nc.tensor.matmul(
                        out=h_ps[:],
                        lhsT=xt[:, k, :],
                        rhs=w1_t[:, k, :],
                        start=(k == 0),
                        stop=(k == 2),
                    )
                ot = wp.tile([P, 80], f32, tag="o")
                nc.vector.tensor_copy(out=ot[:, 0:64], in_=h_ps[:])
                tmp = wp.tile([P, 64], f32, tag="tmp")
                nc.vector.tensor_tensor(
                    out=tmp[:], in0=h_ps[:], in1=asrc_t[:], op=mybir.AluOpType.mult
                )
                nc.vector.reduce_sum(
                    out=ot[:, 64:72],
                    in_=tmp[:].rearrange("p (h d) -> p h d", d=8),
                    axis=mybir.AxisListType.X,
                )
                nc.vector.tensor_tensor(
                    out=tmp[:], in0=h_ps[:], in1=adst_t[:], op=mybir.AluOpType.mult
                )
                nc.vector.reduce_sum(
                    out=ot[:, 72:80],
                    in_=tmp[:].rearrange("p (h d) -> p h d", d=8),
                    axis=mybir.AxisListType.X,
                )
                nc.sync.dma_start(out=out[t * P : (t + 1) * P, :], in_=ot[:])
    nc.finalize()
    return nc


# ------------------------------------------------------------- launch B prog
def _build_B(G):
    """Layer-1 edge pass + b1 + ELU + W2aug matmul -> g2 rows [NPAD, 66]."""
    nc = bacc.Bacc(None, target_bir_lowering=False)
    tot = int((P * G).sum())
    ge = nc.dram_tensor("ge", [tot * 72], mybir.dt.float32, kind="ExternalInput")
    edst = nc.dram_tensor("edst", [NPAD, 8], mybir.dt.float32, kind="ExternalInput")
    b1 = nc.dram_tensor("b1", [64], mybir.dt.float32, kind="ExternalInput")
    w2aug = nc.dram_tensor("w2aug", [64, 66], mybir.dt.float32, kind="ExternalInput")
    badj = nc.dram_tensor("badj", [66], mybir.dt.float32, kind="ExternalInput")
    out = nc.dram_tensor("g2", [NPAD, 66], mybir.dt.float32, kind="ExternalOutput")

    f32 = mybir.dt.float32
    AT = mybir.ActivationFunctionType
    OP = mybir.AluOpType
    with tile.TileContext(nc) as tc:
        with (
            tc.tile_pool(name="const", bufs=1) as cp,
            tc.tile_pool(name="gin", bufs=3) as gp,
            tc.tile_pool(name="work", bufs=2) as wp,
            tc.tile_pool(name="outp", bufs=3) as op_,
            tc.tile_pool(name="psum", bufs=2, space="PSUM") as pp,
        ):
            iden = cp.tile([P, P], f32)
            make_identity(nc, iden[:])
            edst_t = cp.tile([P, NT * 8], f32)
            nc.sync.dma_start(
                out=edst_t[:],
                in_=bass.AP(tensor=edst, offset=0,
                            ap=[[8, P], [8 * P, NT], [1, 8]]),
            )
            b1_t = cp.tile([P, 64], f32)
            nc.sync.dma_start(
                out=b1_t[:],
                in_=bass.AP(tensor=b1, offset=0, ap=[[0, P], [1, 64]]),
            )
            w2_t = cp.tile([64, 66], f32)
            nc.sync.dma_start(out=w2_t[:], in_=w2aug[:, :])
            badj_t = cp.tile([P, 66], f32)
            nc.sync.dma_start(
                out=badj_t[:],
                in_=bass.AP(tensor=badj, offset=0, ap=[[0, P], [1, 66]]),
            )
            off = 0
            for t in range(NT):
                g = int(G[t])
                gt = gp.tile([P, g * 72], f32, tag="g")
                nc.sync.dma_start(
                    out=gt[:],
                    in_=bass.AP(tensor=ge, offset=off,
                                ap=[[g * 72, P], [1, g * 72]]),
                )
                off += P * g * 72
                gap = gt[:]
                base = [gap.ap[0][0], P]

                def gv(o, dims):
                    return bass.AP(tensor=gap.tensor, offset=gap.offset + o,
                                   ap=[base] + dims)

                # e_sum[p, h*g+gi] = ge_esrc[p, gi, h] + edst[p, t, h]
                es = wp.tile([P, 8 * g], f32, tag="es")
                eap = es[:]

                def ev(o, dims):
                    return bass.AP(tensor=eap.tensor, offset=eap.offset + o,
                                   ap=[[eap.ap[0][0], P]] + dims)

                nc.vector.tensor_tensor(
                    out=ev(0, [[1, g], [g, 8]]),
                    in0=gv(64, [[72, g], [1, 8]]),
                    in1=bass.AP(tensor=edst_t[:].tensor,
                                offset=edst_t[:].offset + t * 8,
                                ap=[[edst_t[:].ap[0][0], P], [0, g], [1, 8]]),
                    op=OP.add,
                )
                w_t = wp.tile([P, 8 * g], f32, tag="w")
                nc.scalar.activation(out=w_t[:], in_=es[:], func=AT.Lrelu, alpha=0.2)
                nc.scalar.activation(out=w_t[:], in_=w_t[:], func=AT.Exp)
                wap = w_t[:]

                def wv(o, dims):
                    return bass.AP(tensor=wap.tensor, offset=wap.offset + o,
                                   ap=[[wap.ap[0][0], P]] + dims)

                den = wp.tile([P, 8], f32, tag="den")
                nc.vector.reduce_sum(
                    out=den[:], in_=wv(0, [[g, 8], [1, g]]), axis=mybir.AxisListType.X
                )
                nc.vector.tensor_scalar_max(out=den[:], in0=den[:], scalar1=1e-30)
                rec = wp.tile([P, 8], f32, tag="rec")
                nc.vector.reciprocal(out=rec[:], in_=den[:])
                # ws[p, (h*8+d)*g+gi] = ge_h[p, gi, h, d] * w[p, h, gi]
                ws = wp.tile([P, 64 * g], f32, tag="ws")
                wsap = ws[:]
                nc.vector.tensor_tensor(
                    out=bass.AP(tensor=wsap.tensor, offset=wsap.offset,
                                ap=[[wsap.ap[0][0], P], [1, g], [8 * g, 8], [g, 8]]),
                    in0=gv(0, [[72, g], [8, 8], [1, 8]]),
                    in1=wv(0, [[1, g], [g, 8], [0, 8]]),
                    op=OP.mult,
                )
                o1 = wp.tile([P, 64], f32, tag="o1")
                nc.vector.reduce_sum(
                    out=o1[:],
                    in_=bass.AP(tensor=wsap.tensor, offset=wsap.offset,
                                ap=[[wsap.ap[0][0], P], [8 * g, 8], [g, 8], [1, g]]),
                    axis=mybir.AxisListType.X,
                )
                recap = rec[:]
                nc.vector.tensor_tensor(
                    out=o1[:], in0=o1[:],
                    in1=bass.AP(tensor=recap.tensor, offset=recap.offset,
                                ap=[[recap.ap[0][0], P], [1, 8], [0, 8]]),
                    op=OP.mult,
                )
                nc.vector.tensor_tensor(out=o1[:], in0=o1[:], in1=b1_t[:], op=OP.add)
                # z' = relu(zp) + exp(min(zp,0))  (= elu(zp)+1)
                m = wp.tile([P, 64], f32, tag="m")
                nc.vector.tensor_scalar_min(out=m[:], in0=o1[:], scalar1=0.0)
                nc.scalar.activation(out=m[:], in_=m[:], func=AT.Exp)
                z1 = wp.tile([P, 64], f32, tag="z1")
                nc.vector.scalar_tensor_tensor(
                    out=z1[:], in0=o1[:], scalar=0.0, in1=m[:],
                    op0=OP.max, op1=OP.add,
                )
                zT_ps = pp.tile([64, P], f32, tag="zT")
                nc.tensor.transpose(out=zT_ps[:], in_=z1[:], identity=iden[:])
                zT = wp.tile([64, P], f32, tag="zTs")
                nc.vector.tensor_copy(out=zT[:], in_=zT_ps[:])
                h2_ps = pp.tile([P, 66], f32, tag="h2")
                nc.tensor.matmul(out=h2_ps[:], lhsT=zT[:], rhs=w2_t[:],
                                 start=True, stop=True)
                g2t = op_.tile([P, 66], f32, tag="g2t")
                nc.vector.tensor_tensor(out=g2t[:], in0=h2_ps[:], in1=badj_t[:],
                                        op=OP.add)
                nc.sync.dma_start(out=out[t * P : (t + 1) * P, :], in_=g2t[:])
    nc.finalize()
    return nc


# ------------------------------------------------------------- launch C prog
def _build_C(G):
    """Layer-2 edge pass + b2 + log_softmax -> [NPAD, 64]."""
    nc = bacc.Bacc(None, target_bir_lowering=False)
    tot = int((P * G).sum())
    ge = nc.dram_tensor("ge", [tot * 65], mybir.dt.float32, kind="ExternalInput")
    edst = nc.dram_tensor("edst", [NPAD], mybir.dt.float32, kind="ExternalInput")
    b2 = nc.dram_tensor("b2", [64], mybir.dt.float32, kind="ExternalInput")
    out = nc.dram_tensor("res", [NPAD, 64], mybir.dt.float32, kind="ExternalOutput")

    f32 = mybir.dt.float32
    AT = mybir.ActivationFunctionType
    OP = mybir.AluOpType
    with tile.TileContext(nc) as tc:
        with (
            tc.tile_pool(name="const", bufs=1) as cp,
            tc.tile_pool(name="gin", bufs=3) as gp,
            tc.tile_pool(name="work", bufs=2) as wp,
            tc.tile_pool(name="outp", bufs=3) as op_,
        ):
            edst_t = cp.tile([P, NT], f32)
            nc.sync.dma_start(
                out=edst_t[:],
                in_=bass.AP(tensor=edst, offset=0, ap=[[1, P], [P, NT]]),
            )
            b2_t = cp.tile([P, 64], f32)
            nc.sync.dma_start(
                out=b2_t[:],
                in_=bass.AP(tensor=b2, offset=0, ap=[[0, P], [1, 64]]),
            )
            off = 0
            for t in range(NT):
                g = int(G[t])
                gt = gp.tile([P, g * 65], f32, tag="g")
                nc.sync.dma_start(
                    out=gt[:],
                    in_=bass.AP(tensor=ge, offset=off,
                                ap=[[g * 65, P], [1, g * 65]]),
                )
                off += P * g * 65
                gap = gt[:]
                base = [gap.ap[0][0], P]

                def gv(o, dims):
                    return bass.AP(tensor=gap.tensor, offset=gap.offset + o,
                                   ap=[base] + dims)

                es = wp.tile([P, g], f32, tag="es")
                nc.vector.tensor_tensor(
                    out=es[:],
                    in0=gv(64, [[65, g]]),
                    in1=bass.AP(tensor=edst_t[:].tensor,
                                offset=edst_t[:].offset + t,
                                ap=[[edst_t[:].ap[0][0], P], [0, g]]),
                    op=OP.add,
                )
                w_t = wp.tile([P, g], f32, tag="w")
                nc.scalar.activation(out=w_t[:], in_=es[:], func=AT.Lrelu, alpha=0.2)
                den = wp.tile([P, 1], f32, tag="den")
                nc.scalar.activation(out=w_t[:], in_=w_t[:], func=AT.Exp,
                                     accum_out=den[:])
                nc.vector.tensor_scalar_max(out=den[:], in0=den[:], scalar1=1e-30)
                rec = wp.tile([P, 1], f32, tag="rec")
                nc.vector.reciprocal(out=rec[:], in_=den[:])
                ws = wp.tile([P, 64 * g], f32, tag="ws")
                wsap = ws[:]
                wap = w_t[:]
                nc.vector.tensor_tensor(
                    out=bass.AP(tensor=wsap.tensor, offset=wsap.offset,
                                ap=[[wsap.ap[0][0], P], [1, g], [g, 64]]),
                    in0=gv(0, [[65, g], [1, 64]]),
                    in1=bass.AP(tensor=wap.tensor, offset=wap.offset,
                                ap=[[wap.ap[0][0], P], [1, g], [0, 64]]),
                    op=OP.mult,
                )
                o1 = wp.tile([P, 64], f32, tag="o1")
                nc.vector.reduce_sum(
                    out=o1[:],
                    in_=bass.AP(tensor=wsap.tensor, offset=wsap.offset,
                                ap=[[wsap.ap[0][0], P], [g, 64], [1, g]]),
                    axis=mybir.AxisListType.X,
                )
                z = wp.tile([P, 64], f32, tag="z")
                nc.vector.scalar_tensor_tensor(
                    out=z[:], in0=o1[:], scalar=rec[:, 0:1], in1=b2_t[:],
                    op0=OP.mult, op1=OP.add,
                )
                # log_softmax
                nmx = wp.tile([P, 1], f32, tag="nmx")
                nc.vector.tensor_reduce(
                    out=nmx[:], in_=z[:], axis=mybir.AxisListType.X,
                    op=OP.max, negate=True,
                )
                ex = wp.tile([P, 64], f32, tag="ex")
                ssum = wp.tile([P, 1], f32, tag="ssum")
                nc.scalar.activation(out=ex[:], in_=z[:], func=AT.Exp,
                                     bias=nmx[:, 0:1], scale=1.0,
                                     accum_out=ssum[:])
                lse = wp.tile([P, 1], f32, tag="lse")
                nc.scalar.activation(out=lse[:], in_=ssum[:], func=AT.Ln)
                ot = op_.tile([P, 64], f32, tag="ot")
                lap = lse[:]
                nc.vector.scalar_tensor_tensor(
                    out=ot[:], in0=z[:], scalar=nmx[:, 0:1],
                    in1=bass.AP(tensor=lap.tensor, offset=lap.offset,
                                ap=[[lap.ap[0][0], P], [0, 64]]),
                    op0=OP.add, op1=OP.subtract,
                )
                nc.sync.dma_start(out=out[t * P : (t + 1) * P, :], in_=ot[:])
    nc.finalize()
    return nc


# ------------------------------------------------------------------- driver
def _get_programs(G):
    key = tuple(int(g) for g in G)
    if key not in _cache:
        _cache[key] = (_build_A(), _build_B(G), _build_C(G))
    return _cache[key]


def kernel(x, edge_index, W1, att_src1, att_dst1, b1, W2, att_src2, att_dst2, b2,
           _timings=None):
    import time as _time

    x = np.asarray(x, dtype=np.float32)
    W1 = np.asarray(W1, dtype=np.float32)
    order_all, pos, G, A, tot = _host_prep(np.asarray(edge_index))
    ncA, ncB, ncC = _get_programs(G)

    # ---- launch A inputs
    w1p = np.zeros((FK, 64), np.float32)
    w1p[:F_IN] = W1
    asrc = np.asarray(att_src1, np.float32).ravel()
    adst = np.asarray(att_dst1, np.float32).ravel()
    xpad = np.vstack([x, np.zeros((1, F_IN), np.float32)])
    in_A = []
    for c in range(NCORES):
        xa = xpad[np.where(order_all[c] >= 0, order_all[c], N)]  # [NPAD, 300]
        xT = np.zeros((FK, NPAD), np.float32)
        xT[:F_IN] = xa.T
        in_A.append({"xT": xT, "w1": w1p, "asrc": asrc, "adst": adst})

    t0 = _time.perf_counter()
    resA = run_bass_kernel_spmd(ncA, in_A, core_ids=list(range(NCORES)))
    tA = _time.perf_counter() - t0

    h1x = np.concatenate([r["h1x"] for r in resA.results], axis=0)  # [8*NPAD, 80]
    tab1 = np.vstack([h1x[:, :72],
                      np.full((1, 72), 0, np.float32)])
    tab1[-1, 64:72] = SENT_BIG

    # ---- launch B inputs
    W2 = np.asarray(W2, np.float32)
    w2aug = np.concatenate(
        [W2, (W2 @ np.asarray(att_src2, np.float32).ravel())[:, None],
         (W2 @ np.asarray(att_dst2, np.float32).ravel())[:, None]], axis=1)
    badj = -w2aug.sum(axis=0).astype(np.float32)
    b1 = np.asarray(b1, np.float32)
    in_B = []
    for c in range(NCORES):
        ge = tab1[A[c]].ravel()
        in_B.append({"ge": ge, "edst": h1x[c * NPAD:(c + 1) * NPAD, 72:80].copy(),
                     "b1": b1, "w2aug": w2aug, "badj": badj})

    t0 = _time.perf_counter()
    resB = run_bass_kernel_spmd(ncB, in_B, core_ids=list(range(NCORES)))
    tB = _time.perf_counter() - t0

    g2 = np.concatenate([r["g2"] for r in resB.results], axis=0)  # [8*NPAD, 66]
    tab2 = np.vstack([g2[:, :65], np.zeros((1, 65), np.float32)])
    tab2[-1, 64] = SENT_BIG

    # ---- launch C inputs
    b2 = np.asarray(b2, np.float32)
    in_C = []
    for c in range(NCORES):
        ge = tab2[A[c]].ravel()
        in_C.append({"ge": ge, "edst": g2[c * NPAD:(c + 1) * NPAD, 65].copy(),
                     "b2": b2})

    t0 = _time.perf_counter()
    resC = run_bass_kernel_spmd(ncC, in_C, core_ids=list(range(NCORES)))
    tC = _time.perf_counter() - t0

    out = np.empty((N, 64), np.float32)
    for c in range(NCORES):
        valid = order_all[c] >= 0
        out[order_all[c][valid]] = resC.results[c]["res"][valid]
    if _timings is not None:
        _timings.update({"A": tA, "B": tB, "C": tC})
    return out
